# revision 15
# baseline (speedup 1.0000x reference)
"""MultiHeadAttention (tanh-capped logits, key-padding mask) on 8 Trainium2 cores.

Problem: B=4, S=2048, E=1024, H=16, DH=64.
  u = (Q K^T) * scale / sqrt(DH); logits = tanh(u) * exp(log_C)
  logits[masked] = -inf; attn = softmax(logits); out = (attn V) @ W_out.T

Sharding: core c handles batch b=c//2 and heads [8*(c%2), 8*(c%2)+8).
Each core computes a partial y^T = sum over its 8 heads of W_out-slice @ o_h^T;
the host sums the 2 cores of each batch and transposes.

Key optimizations over the dense version:
  * Mask compression: the key-padding mask is shared across heads and queries,
    so the host gathers only the kept keys (~half) for K and V. Attention over
    the gathered keys is mathematically exact; padded key columns produce
    p=exp(0)=1 but contribute nothing (V rows and the denominator ones-column
    are zero there). This roughly halves QK/PV matmul work and the ACT-engine
    tanh+exp passes.
  * fp16 operands everywhere (measured end-to-end rel_fro ~1.5e-3, tolerance
    2e-2); halves SBUF footprint and DMA bytes, fastest PE mode.
  * NKC (key tile count) chosen at runtime from the actual mask; program
    compiled per (scale, gain, NKC).
  * Projection pairs heads on the PE contraction dim (K=128): even heads use
    V-augmentation [V|1] with PV output at psum partitions 0-64, odd heads
    [1|V] at partitions 63-127, so each pair's normalized outputs stack into
    one [128, QT] tile and the output projection runs at K=128.

Device pipeline per (q-tile of 512, head):
  logits^T[k, q] tiles via PE, tanh on ACT (scale folded), exp on ACT
  (gain folded) in one pass over the head's P^T, P^T @ V-aug accumulated on
  PE -> [o^T; r], normalize via reciprocal_approx_fast + gpsimd
  partition_broadcast + DVE multiply, projection accumulated over head pairs
  on PE.
"""
import math
import os

os.environ.setdefault("JAX_COMPILATION_CACHE_DIR", "/tmp/jax_comp_cache")

import numpy as np

import concourse.bass as bass
import concourse.tile as tile
from concourse import bacc, mybir
from concourse import bass_utils
from concourse.bass_interp import get_hw_module

F32 = mybir.dt.float32
F16 = mybir.dt.float16

B, S, E, H, DH = 4, 2048, 1024, 16, 64
N_CORES = 8
HPC = 8  # heads per core
QT = 512  # q tile
NQT = S // QT  # 4

_CACHE = {}


def _build(scale_eff: float, gain: float, nkc: int, reps: int = 1):
    """nkc: number of 128-wide key tiles after mask compression."""
    kp = nkc * 128
    nc = bacc.Bacc(
        "TRN2",
        target_bir_lowering=False,
        debug=False,
        enable_asserts=True,
        num_devices=N_CORES,
    )
    kT_d = nc.dram_tensor("kT", [128, 4, kp], F16, kind="ExternalInput").ap()
    qT_d = nc.dram_tensor("qT", [128, 4, S], F16, kind="ExternalInput").ap()
    v_d = nc.dram_tensor("vA", [128, HPC, nkc, DH + 1], F16, kind="ExternalInput").ap()
    woT_d = nc.dram_tensor("woT", [128, 4, E], F16, kind="ExternalInput").ap()
    yT_d = nc.dram_tensor("yT", [E, S], F16, kind="ExternalOutput").ap()

    # tanh chunks of k-tiles: triples + remainder (bigger ACT instructions
    # amortize the fixed SBUF/PSUM access latency per instruction)
    CW = 3
    chunks = [(i, min(i + CW, nkc)) for i in range(0, nkc, CW)]

    with tile.TileContext(nc) as tc:
        with (
            tc.tile_pool(name="resident", bufs=1) as res_pool,
            tc.tile_pool(name="pt", bufs=2) as pt_pool,
            tc.tile_pool(name="onorm", bufs=9) as onorm_pool,
            tc.tile_pool(name="rspool", bufs=1) as rs_pool,
            tc.tile_pool(name="yout", bufs=2) as y_pool,
            tc.tile_pool(name="qk_ps", bufs=2, space="PSUM") as qk_ps,
            tc.tile_pool(name="pvj_ps", bufs=2, space="PSUM") as pvj_ps,
        ):
            # Spread resident loads over queues; first QK needs only kT pair 0
            # and the first q-tile, so those go first on the sync queue.
            kT_sb = res_pool.tile([128, 4, kp], F16, tag="kT")
            nc.sync.dma_start(out=kT_sb[:, 0, 0:256], in_=kT_d[:, 0, 0:256])
            qT_sb = res_pool.tile([128, 4, S], F16, tag="qT")
            nc.sync.dma_start(out=qT_sb[:, 0, 0:QT], in_=qT_d[:, 0, 0:QT])
            nc.sync.dma_start(out=kT_sb[:, 0, 256:kp], in_=kT_d[:, 0, 256:kp])
            nc.sync.dma_start(out=qT_sb[:, 0, QT:S], in_=qT_d[:, 0, QT:S])
            v_sb = res_pool.tile([128, HPC, nkc, DH + 1], F16, tag="v")
            for h in range(HPC):
                nc.gpsimd.dma_start(out=v_sb[:, h], in_=v_d[:, h])
            woT_sb = res_pool.tile([128, 4, E], F16, tag="woT")
            for j in range(1, 4):
                nc.gpsimd.dma_start(out=kT_sb[:, j], in_=kT_d[:, j])
                nc.gpsimd.dma_start(out=qT_sb[:, j], in_=qT_d[:, j])
            nc.gpsimd.dma_start(out=woT_sb, in_=woT_d)

            def emit_proj_eo(onorm_list, qt_idx, eo, final=False):
                if final:
                    # after all QK work: borrow the idle qk pool so the
                    # tail projection double-buffers
                    py_full = qk_ps.tile([128, CW * QT], F32, tag="qk", name="pyf")
                    py = py_full[:, 0:QT]
                else:
                    py = pvj_ps.tile([128, QT], F32, tag="pvj")
                for jj in range(4):
                    nc.tensor.matmul(
                        py,
                        lhsT=woT_sb[:, jj, eo * 128 : (eo + 1) * 128],
                        rhs=onorm_list[jj],
                        start=(jj == 0),
                        stop=(jj == 3),
                    )
                y_t = y_pool.tile([128, QT], F16, tag="y")
                nc.vector.tensor_copy(out=y_t, in_=py)
                nc.sync.dma_start(
                    out=yT_d[
                        eo * 128 : (eo + 1) * 128,
                        qt_idx * QT : (qt_idx + 1) * QT,
                    ],
                    in_=y_t,
                )

            pending = None  # (onorm_tiles, qt) awaiting projection emission
            state = {"onorm_tiles": [], "onorm_cur": None, "pending": None}

            def emit_pv(qt, h, pt_t):
                """PV + normalize for head h of q-tile qt (issued one head
                late so the PE's QK of head h+1 overlaps ACT's exp of h)."""
                half = h % 2
                # PV output [o^T; r] at psum partitions 0-64 (matmul psum
                # base must be 0/32/64). Normalized even-head output lands
                # at partitions 0-63 of the pair tile directly; odd-head
                # output is normalized into a scratch tile and DMA-moved to
                # partitions 64-127 so the projection runs at K=128.
                po_full = pvj_ps.tile([128, QT], F32, tag="pvj", name="po")
                po_t = po_full[0 : DH + 1]
                for kt in range(nkc):
                    nc.tensor.matmul(
                        po_t,
                        lhsT=v_sb[:, h, kt, :],
                        rhs=pt_t[:, kt * QT : (kt + 1) * QT],
                        start=(kt == 0),
                        stop=(kt == nkc - 1),
                    )
                # r lives on psum partition 64; engines are partition-locked,
                # so copy it to SBUF, DMA-move to partition 0, recip there,
                # then gpsimd-broadcast to partitions 0-63.
                rs = rs_pool.tile([DH + 1, QT], F32, tag=f"rs{half}")
                nc.vector.tensor_copy(
                    out=rs[DH : DH + 1, :], in_=po_t[DH : DH + 1, :]
                )
                mv = rs_pool.tile([1, QT], F32, tag=f"mv{half}")
                nc.sync.dma_start(out=mv, in_=rs[DH : DH + 1, :])
                rec1 = rs_pool.tile([1, QT], F32, tag=f"rec{half}")
                nc.vector.reciprocal_approx_fast(out=rec1, in_=mv)
                rb = rs_pool.tile([64, QT], F32, tag=f"rb{half}")
                nc.gpsimd.partition_broadcast(rb, rec1)
                if half == 0:
                    onorm = onorm_pool.tile([128, QT], F16, tag="on")
                    state["onorm_cur"] = onorm
                    nc.vector.tensor_mul(out=onorm[0:64], in0=po_t[0:DH, :], in1=rb)
                else:
                    onorm = state["onorm_cur"]
                    o_scr = rs_pool.tile([64, QT], F16, tag="oscr")
                    nc.vector.tensor_mul(out=o_scr, in0=po_t[0:DH, :], in1=rb)
                    nc.sync.dma_start(out=onorm[64:128], in_=o_scr)
                    state["onorm_tiles"].append(onorm)
                if h == HPC - 1:
                    state["pending"] = (state["onorm_tiles"], qt)
                    state["onorm_tiles"] = []

            prev = None  # (qt, h, pt_t) whose PV is deferred one head
            for qt in [q for _ in range(reps) for q in range(NQT)]:
                for h in range(HPC):
                    j, half = h // 2, h % 2
                    lo = 64 * half
                    pt_t = pt_pool.tile([128, nkc * QT], F16, tag="pt")
                    for ci, (c0, c1) in enumerate(chunks):
                        nk = c1 - c0
                        ps = qk_ps.tile([128, CW * QT], F32, tag="qk", name="ps")
                        for w in range(nk):
                            kt = c0 + w
                            nc.tensor.matmul(
                                ps[:, w * QT : (w + 1) * QT],
                                lhsT=kT_sb[lo : lo + 64, j, kt * 128 : (kt + 1) * 128],
                                rhs=qT_sb[lo : lo + 64, j, qt * QT : (qt + 1) * QT],
                                start=True,
                                stop=True,
                            )
                        nc.scalar.activation(
                            out=pt_t[:, c0 * QT : c1 * QT],
                            in_=ps[:, 0 : nk * QT],
                            func=mybir.ActivationFunctionType.Tanh,
                            scale=scale_eff,
                        )
                        if ci == 0:
                            # Divert the PE to the previous head's PV and the
                            # previous q-tile's projection only after this
                            # head's first QK chunk, so ACT's next tanh input
                            # is ready the moment its exp finishes.
                            if prev is not None:
                                emit_pv(*prev)
                            if state["pending"] is not None and h >= 1:
                                ol, pqt = state["pending"]
                                emit_proj_eo(ol, pqt, h - 1)
                                if h == HPC - 1:
                                    emit_proj_eo(ol, pqt, 7)
                                    state["pending"] = None
                    nc.scalar.activation(
                        out=pt_t,
                        in_=pt_t,
                        func=mybir.ActivationFunctionType.Exp,
                        scale=gain,
                    )
                    prev = (qt, h, pt_t)
            emit_pv(*prev)
            ol, pqt = state["pending"]
            for eo in range(8):
                emit_proj_eo(ol, pqt, eo, final=True)

    nc.compile()
    return nc


def _get_nc(scale_eff: float, gain: float, nkc: int):
    key = (round(scale_eff, 12), round(gain, 12), nkc)
    if key not in _CACHE:
        _CACHE[key] = _build(scale_eff, gain, nkc)
    return _CACHE[key]


def _prep_core_inputs(query, key, value, mask, W_out, nkc):
    """Host-side mask compression + sharding + layout. List of 8 in_maps."""
    kp = nkc * 128
    keep = ~mask[:, 0, :]  # [B, S]; True in mask = drop

    per_batch = []
    for b in range(B):
        idx = np.flatnonzero(keep[b])
        nk = len(idx)
        k_g = np.zeros((kp, E), dtype=np.float32)
        k_g[:nk] = key[b][idx]
        v_g = np.zeros((kp, E), dtype=np.float32)
        v_g[:nk] = value[b][idx]
        ones_g = np.zeros((kp, 1, 1), dtype=np.float32)
        ones_g[:nk] = 1.0
        per_batch.append((k_g, v_g, ones_g))

    in_maps = []
    for c in range(N_CORES):
        b, hh = c // 2, c % 2
        hsl = slice(8 * hh, 8 * hh + 8)
        k_g, v_g, ones_g = per_batch[b]

        k4 = k_g.reshape(kp, H, DH)[:, hsl, :]  # [kp, 8, 64]
        kT = np.ascontiguousarray(
            k4.transpose(1, 2, 0).reshape(4, 128, kp).transpose(1, 0, 2)
        ).astype(np.float16)
        q4 = query[b].reshape(S, H, DH)[:, hsl, :]
        qT = np.ascontiguousarray(
            q4.transpose(1, 2, 0).reshape(4, 128, S).transpose(1, 0, 2)
        ).astype(np.float16)
        woT = np.ascontiguousarray(
            W_out.reshape(E, H, DH)[:, hsl, :]
            .transpose(1, 2, 0)
            .reshape(4, 128, E)
            .transpose(1, 0, 2)
        ).astype(np.float16)  # [128, 4, E], head pairs stacked on partitions

        v4 = v_g.reshape(kp, H, DH)[:, hsl, :]  # [kp, 8, 64]
        ones_b = np.broadcast_to(ones_g, (kp, HPC, 1))
        aug = np.concatenate([v4, ones_b], axis=2)  # [kp, 8, 65]
        vA = np.ascontiguousarray(
            aug.reshape(nkc, 128, HPC, DH + 1).transpose(1, 2, 0, 3)
        ).astype(np.float16)

        in_maps.append({"kT": kT, "qT": qT, "vA": vA, "woT": woT})
    return in_maps


def kernel(query, key, value, mask, W_out, scale, log_C) -> np.ndarray:
    query = np.asarray(query, dtype=np.float32)
    key = np.asarray(key, dtype=np.float32)
    value = np.asarray(value, dtype=np.float32)
    mask = np.asarray(mask)
    W_out = np.asarray(W_out, dtype=np.float32)
    scale_eff = float(np.asarray(scale)) / math.sqrt(DH)
    gain = float(np.exp(np.float64(np.asarray(log_C))))

    keep_counts = (~mask[:, 0, :]).sum(axis=1)
    nkc = max(1, int(math.ceil(int(keep_counts.max()) / 128)))

    nc = _get_nc(scale_eff, gain, nkc)
    in_maps = _prep_core_inputs(query, key, value, mask, W_out, nkc)

    old = nc.m
    nc.m = get_hw_module(nc.m)
    try:
        res = bass_utils.run_bass_kernel_spmd(
            nc, in_maps, core_ids=list(range(N_CORES))
        )
    finally:
        nc.m = old

    out = np.empty((B, S, E), dtype=np.float32)
    for b in range(B):
        yT = res.results[2 * b]["yT"].astype(np.float32) + res.results[
            2 * b + 1
        ]["yT"].astype(np.float32)
        out[b] = yT.T
    return out


# revision 16
# speedup vs baseline: 1.2327x; 1.2327x over previous
"""MultiHeadAttention (tanh-capped logits, key-padding mask) on 8 Trainium2 cores.

Problem: B=4, S=2048, E=1024, H=16, DH=64.
  u = (Q K^T) * scale / sqrt(DH); logits = tanh(u) * exp(log_C)
  logits[masked] = -inf; attn = softmax(logits); out = (attn V) @ W_out.T

Sharding: core c handles batch b=c//2 and heads [8*(c%2), 8*(c%2)+8).
Each core computes a partial y^T = sum over its 8 heads of W_out-slice @ o_h^T;
the host sums the 2 cores of each batch and transposes.

Key optimizations over the dense version:
  * Mask compression: the key-padding mask is shared across heads and queries,
    so the host gathers only the kept keys (~half) for K and V. Attention over
    the gathered keys is mathematically exact; padded key columns produce
    p=exp(0)=1 but contribute nothing (V rows and the denominator ones-column
    are zero there). This roughly halves QK/PV matmul work and the ACT-engine
    tanh+exp passes.
  * fp16 operands everywhere (measured end-to-end rel_fro ~1.5e-3, tolerance
    2e-2); halves SBUF footprint and DMA bytes, fastest PE mode.
  * NKC (key tile count) chosen at runtime from the actual mask; program
    compiled per (scale, gain, NKC).
  * Projection pairs heads on the PE contraction dim (K=128): even heads use
    V-augmentation [V|1] with PV output at psum partitions 0-64, odd heads
    [1|V] at partitions 63-127, so each pair's normalized outputs stack into
    one [128, QT] tile and the output projection runs at K=128.

Device pipeline per (q-tile of 512, head):
  logits^T[k, q] tiles via PE, tanh on ACT (scale folded), exp on ACT
  (gain folded) in one pass over the head's P^T, P^T @ V-aug accumulated on
  PE -> [o^T; r], normalize via reciprocal_approx_fast + gpsimd
  partition_broadcast + DVE multiply, projection accumulated over head pairs
  on PE.
"""
import math
import os

os.environ.setdefault("JAX_COMPILATION_CACHE_DIR", "/tmp/jax_comp_cache")

import numpy as np

import concourse.bass as bass
import concourse.tile as tile
from concourse import bacc, mybir
from concourse import bass_utils
from concourse.bass_interp import get_hw_module

F32 = mybir.dt.float32
F16 = mybir.dt.float16

B, S, E, H, DH = 4, 2048, 1024, 16, 64
N_CORES = 8
HPC = 8  # heads per core
QT = 512  # q tile
NQT = S // QT  # 4

_CACHE = {}


def _build(scale_eff: float, gain: float, nkc: int, reps: int = 1):
    """nkc: number of 128-wide key tiles after mask compression."""
    kp = nkc * 128
    nc = bacc.Bacc(
        "TRN2",
        target_bir_lowering=False,
        debug=False,
        enable_asserts=True,
        num_devices=N_CORES,
    )
    kT_d = nc.dram_tensor("kT", [128, 4, kp], F16, kind="ExternalInput").ap()
    qT_d = nc.dram_tensor("qT", [128, 4, S], F16, kind="ExternalInput").ap()
    v_d = nc.dram_tensor("vA", [128, HPC, nkc, DH + 1], F16, kind="ExternalInput").ap()
    woT_d = nc.dram_tensor("woT", [128, 4, E], F16, kind="ExternalInput").ap()
    yT_d = nc.dram_tensor("yT", [E, S], F16, kind="ExternalOutput").ap()

    # tanh chunks of k-tiles: triples + remainder (bigger ACT instructions
    # amortize the fixed SBUF/PSUM access latency per instruction)
    CW = 3
    chunks = [(i, min(i + CW, nkc)) for i in range(0, nkc, CW)]

    with tile.TileContext(nc) as tc:
        with (
            tc.tile_pool(name="resident", bufs=1) as res_pool,
            tc.tile_pool(name="pt", bufs=2) as pt_pool,
            tc.tile_pool(name="onorm", bufs=9) as onorm_pool,
            tc.tile_pool(name="rspool", bufs=1) as rs_pool,
            tc.tile_pool(name="yout", bufs=2) as y_pool,
            tc.tile_pool(name="qk_ps", bufs=2, space="PSUM") as qk_ps,
            tc.tile_pool(name="pvj_ps", bufs=2, space="PSUM") as pvj_ps,
        ):
            # Spread resident loads over queues; first QK needs only kT pair 0
            # and the first q-tile, so those go first on the sync queue.
            kT_sb = res_pool.tile([128, 4, kp], F16, tag="kT")
            nc.sync.dma_start(out=kT_sb[:, 0, 0:256], in_=kT_d[:, 0, 0:256])
            qT_sb = res_pool.tile([128, 4, S], F16, tag="qT")
            nc.sync.dma_start(out=qT_sb[:, 0, 0:QT], in_=qT_d[:, 0, 0:QT])
            nc.sync.dma_start(out=kT_sb[:, 0, 256:kp], in_=kT_d[:, 0, 256:kp])
            nc.sync.dma_start(out=qT_sb[:, 0, QT:S], in_=qT_d[:, 0, QT:S])
            v_sb = res_pool.tile([128, HPC, nkc, DH + 1], F16, tag="v")
            for h in range(HPC):
                nc.gpsimd.dma_start(out=v_sb[:, h], in_=v_d[:, h])
            woT_sb = res_pool.tile([128, 4, E], F16, tag="woT")
            for j in range(1, 4):
                nc.gpsimd.dma_start(out=kT_sb[:, j], in_=kT_d[:, j])
                nc.gpsimd.dma_start(out=qT_sb[:, j], in_=qT_d[:, j])
            nc.gpsimd.dma_start(out=woT_sb, in_=woT_d)

            def emit_proj_eo(onorm_list, qt_idx, eo, final=False):
                if final:
                    # after all QK work: borrow the idle qk pool so the
                    # tail projection double-buffers
                    py_full = qk_ps.tile([128, CW * QT], F32, tag="qk", name="pyf")
                    py = py_full[:, 0:QT]
                else:
                    py = pvj_ps.tile([128, QT], F32, tag="pvj")
                for jj in range(4):
                    nc.tensor.matmul(
                        py,
                        lhsT=woT_sb[:, jj, eo * 128 : (eo + 1) * 128],
                        rhs=onorm_list[jj],
                        start=(jj == 0),
                        stop=(jj == 3),
                    )
                y_t = y_pool.tile([128, QT], F16, tag="y")
                nc.vector.tensor_copy(out=y_t, in_=py)
                nc.sync.dma_start(
                    out=yT_d[
                        eo * 128 : (eo + 1) * 128,
                        qt_idx * QT : (qt_idx + 1) * QT,
                    ],
                    in_=y_t,
                )

            pending = None  # (onorm_tiles, qt) awaiting projection emission
            state = {"onorm_tiles": [], "onorm_cur": None, "pending": None}

            def emit_pv(qt, h, pt_t):
                """PV + normalize for head h of q-tile qt (issued one head
                late so the PE's QK of head h+1 overlaps ACT's exp of h)."""
                half = h % 2
                # PV output [o^T; r] at psum partitions 0-64 (matmul psum
                # base must be 0/32/64). Normalized even-head output lands
                # at partitions 0-63 of the pair tile directly; odd-head
                # output is normalized into a scratch tile and DMA-moved to
                # partitions 64-127 so the projection runs at K=128.
                po_full = pvj_ps.tile([128, QT], F32, tag="pvj", name="po")
                po_t = po_full[0 : DH + 1]
                for kt in range(nkc):
                    nc.tensor.matmul(
                        po_t,
                        lhsT=v_sb[:, h, kt, :],
                        rhs=pt_t[:, kt * QT : (kt + 1) * QT],
                        start=(kt == 0),
                        stop=(kt == nkc - 1),
                    )
                # r lives on psum partition 64; engines are partition-locked,
                # so copy it to SBUF, DMA-move to partition 0, recip there,
                # then gpsimd-broadcast to partitions 0-63.
                rs = rs_pool.tile([DH + 1, QT], F32, tag=f"rs{half}")
                nc.vector.tensor_copy(
                    out=rs[DH : DH + 1, :], in_=po_t[DH : DH + 1, :]
                )
                mv = rs_pool.tile([1, QT], F32, tag=f"mv{half}")
                nc.sync.dma_start(out=mv, in_=rs[DH : DH + 1, :])
                rec1 = rs_pool.tile([1, QT], F32, tag=f"rec{half}")
                nc.vector.reciprocal_approx_fast(out=rec1, in_=mv)
                rb = rs_pool.tile([64, QT], F32, tag=f"rb{half}")
                nc.gpsimd.partition_broadcast(rb, rec1)
                if half == 0:
                    onorm = onorm_pool.tile([128, QT], F16, tag="on")
                    state["onorm_cur"] = onorm
                    nc.vector.tensor_mul(out=onorm[0:64], in0=po_t[0:DH, :], in1=rb)
                else:
                    onorm = state["onorm_cur"]
                    o_scr = rs_pool.tile([64, QT], F16, tag="oscr")
                    nc.vector.tensor_mul(out=o_scr, in0=po_t[0:DH, :], in1=rb)
                    nc.sync.dma_start(out=onorm[64:128], in_=o_scr)
                    state["onorm_tiles"].append(onorm)
                if h == HPC - 1:
                    state["pending"] = (state["onorm_tiles"], qt)
                    state["onorm_tiles"] = []

            prev = None  # (qt, h, pt_t) whose PV is deferred one head
            for qt in [q for _ in range(reps) for q in range(NQT)]:
                for h in range(HPC):
                    j, half = h // 2, h % 2
                    lo = 64 * half
                    pt_t = pt_pool.tile([128, nkc * QT], F16, tag="pt")
                    for c0, c1 in chunks:
                        nk = c1 - c0
                        ps = qk_ps.tile([128, CW * QT], F32, tag="qk", name="ps")
                        for w in range(nk):
                            kt = c0 + w
                            nc.tensor.matmul(
                                ps[:, w * QT : (w + 1) * QT],
                                lhsT=kT_sb[lo : lo + 64, j, kt * 128 : (kt + 1) * 128],
                                rhs=qT_sb[lo : lo + 64, j, qt * QT : (qt + 1) * QT],
                                start=True,
                                stop=True,
                            )
                        nc.scalar.activation(
                            out=pt_t[:, c0 * QT : c1 * QT],
                            in_=ps[:, 0 : nk * QT],
                            func=mybir.ActivationFunctionType.Tanh,
                            scale=scale_eff,
                        )
                    nc.scalar.activation(
                        out=pt_t,
                        in_=pt_t,
                        func=mybir.ActivationFunctionType.Exp,
                        scale=gain,
                    )
                    if prev is not None:
                        emit_pv(*prev)
                    prev = (qt, h, pt_t)
                    # Spread the previous q-tile's projection one eo-block
                    # per head stage so it overlaps ACT work instead of
                    # stalling the PE in one burst.
                    if state["pending"] is not None and h >= 1:
                        ol, pqt = state["pending"]
                        emit_proj_eo(ol, pqt, h - 1)
                        if h == HPC - 1:
                            emit_proj_eo(ol, pqt, 7)
                            state["pending"] = None
            emit_pv(*prev)
            ol, pqt = state["pending"]
            for eo in range(8):
                emit_proj_eo(ol, pqt, eo, final=True)

    nc.compile()
    return nc


def _get_nc(scale_eff: float, gain: float, nkc: int):
    key = (round(scale_eff, 12), round(gain, 12), nkc)
    if key not in _CACHE:
        _CACHE[key] = _build(scale_eff, gain, nkc)
    return _CACHE[key]


def _prep_core_inputs(query, key, value, mask, W_out, nkc):
    """Host-side mask compression + sharding + layout. List of 8 in_maps."""
    kp = nkc * 128
    keep = ~mask[:, 0, :]  # [B, S]; True in mask = drop

    per_batch = []
    for b in range(B):
        idx = np.flatnonzero(keep[b])
        nk = len(idx)
        k_g = np.zeros((kp, E), dtype=np.float32)
        k_g[:nk] = key[b][idx]
        v_g = np.zeros((kp, E), dtype=np.float32)
        v_g[:nk] = value[b][idx]
        ones_g = np.zeros((kp, 1, 1), dtype=np.float32)
        ones_g[:nk] = 1.0
        per_batch.append((k_g, v_g, ones_g))

    in_maps = []
    for c in range(N_CORES):
        b, hh = c // 2, c % 2
        hsl = slice(8 * hh, 8 * hh + 8)
        k_g, v_g, ones_g = per_batch[b]

        k4 = k_g.reshape(kp, H, DH)[:, hsl, :]  # [kp, 8, 64]
        kT = np.ascontiguousarray(
            k4.transpose(1, 2, 0).reshape(4, 128, kp).transpose(1, 0, 2)
        ).astype(np.float16)
        q4 = query[b].reshape(S, H, DH)[:, hsl, :]
        qT = np.ascontiguousarray(
            q4.transpose(1, 2, 0).reshape(4, 128, S).transpose(1, 0, 2)
        ).astype(np.float16)
        woT = np.ascontiguousarray(
            W_out.reshape(E, H, DH)[:, hsl, :]
            .transpose(1, 2, 0)
            .reshape(4, 128, E)
            .transpose(1, 0, 2)
        ).astype(np.float16)  # [128, 4, E], head pairs stacked on partitions

        v4 = v_g.reshape(kp, H, DH)[:, hsl, :]  # [kp, 8, 64]
        ones_b = np.broadcast_to(ones_g, (kp, HPC, 1))
        aug = np.concatenate([v4, ones_b], axis=2)  # [kp, 8, 65]
        vA = np.ascontiguousarray(
            aug.reshape(nkc, 128, HPC, DH + 1).transpose(1, 2, 0, 3)
        ).astype(np.float16)

        in_maps.append({"kT": kT, "qT": qT, "vA": vA, "woT": woT})
    return in_maps


def kernel(query, key, value, mask, W_out, scale, log_C) -> np.ndarray:
    query = np.asarray(query, dtype=np.float32)
    key = np.asarray(key, dtype=np.float32)
    value = np.asarray(value, dtype=np.float32)
    mask = np.asarray(mask)
    W_out = np.asarray(W_out, dtype=np.float32)
    scale_eff = float(np.asarray(scale)) / math.sqrt(DH)
    gain = float(np.exp(np.float64(np.asarray(log_C))))

    keep_counts = (~mask[:, 0, :]).sum(axis=1)
    nkc = max(1, int(math.ceil(int(keep_counts.max()) / 128)))

    nc = _get_nc(scale_eff, gain, nkc)
    in_maps = _prep_core_inputs(query, key, value, mask, W_out, nkc)

    old = nc.m
    nc.m = get_hw_module(nc.m)
    try:
        res = bass_utils.run_bass_kernel_spmd(
            nc, in_maps, core_ids=list(range(N_CORES))
        )
    finally:
        nc.m = old

    out = np.empty((B, S, E), dtype=np.float32)
    for b in range(B):
        yT = res.results[2 * b]["yT"].astype(np.float32) + res.results[
            2 * b + 1
        ]["yT"].astype(np.float32)
        out[b] = yT.T
    return out


# revision 17
# speedup vs baseline: 1.2329x; 1.0002x over previous
"""MultiHeadAttention (tanh-capped logits, key-padding mask) on 8 Trainium2 cores.

Problem: B=4, S=2048, E=1024, H=16, DH=64.
  u = (Q K^T) * scale / sqrt(DH); logits = tanh(u) * exp(log_C)
  logits[masked] = -inf; attn = softmax(logits); out = (attn V) @ W_out.T

Sharding: core c handles batch b=c//2 and heads [8*(c%2), 8*(c%2)+8).
Each core computes a partial y^T = sum over its 8 heads of W_out-slice @ o_h^T;
the host sums the 2 cores of each batch and transposes.

Key optimizations over the dense version:
  * Mask compression: the key-padding mask is shared across heads and queries,
    so the host gathers only the kept keys (~half) for K and V. Attention over
    the gathered keys is mathematically exact; padded key columns produce
    p=exp(0)=1 but contribute nothing (V rows and the denominator ones-column
    are zero there). This roughly halves QK/PV matmul work and the ACT-engine
    tanh+exp passes.
  * fp16 operands everywhere (measured end-to-end rel_fro ~1.5e-3, tolerance
    2e-2); halves SBUF footprint and DMA bytes, fastest PE mode.
  * NKC (key tile count) chosen at runtime from the actual mask; program
    compiled per (scale, gain, NKC).
  * Projection pairs heads on the PE contraction dim (K=128): even heads use
    V-augmentation [V|1] with PV output at psum partitions 0-64, odd heads
    [1|V] at partitions 63-127, so each pair's normalized outputs stack into
    one [128, QT] tile and the output projection runs at K=128.

Device pipeline per (q-tile of 512, head):
  logits^T[k, q] tiles via PE, tanh on ACT (scale folded), exp on ACT
  (gain folded) in one pass over the head's P^T, P^T @ V-aug accumulated on
  PE -> [o^T; r], normalize via reciprocal_approx_fast + gpsimd
  partition_broadcast + DVE multiply, projection accumulated over head pairs
  on PE.
"""
import math
import os

os.environ.setdefault("JAX_COMPILATION_CACHE_DIR", "/tmp/jax_comp_cache")

import numpy as np

import concourse.bass as bass
import concourse.tile as tile
from concourse import bacc, mybir
from concourse import bass_utils
from concourse.bass_interp import get_hw_module

F32 = mybir.dt.float32
F16 = mybir.dt.float16

B, S, E, H, DH = 4, 2048, 1024, 16, 64
N_CORES = 8
HPC = 8  # heads per core
QT = 512  # q tile
NQT = S // QT  # 4

_CACHE = {}


def _build(scale_eff: float, gain: float, nkc: int, reps: int = 1):
    """nkc: number of 128-wide key tiles after mask compression."""
    kp = nkc * 128
    nc = bacc.Bacc(
        "TRN2",
        target_bir_lowering=False,
        debug=False,
        enable_asserts=True,
        num_devices=N_CORES,
    )
    kT_d = nc.dram_tensor("kT", [128, 4, kp], F16, kind="ExternalInput").ap()
    qT_d = nc.dram_tensor("qT", [128, 4, S], F16, kind="ExternalInput").ap()
    v_d = nc.dram_tensor("vA", [128, HPC, nkc, DH + 1], F16, kind="ExternalInput").ap()
    woT_d = nc.dram_tensor("woT", [128, 4, E], F16, kind="ExternalInput").ap()
    yT_d = nc.dram_tensor("yT", [E, S], F16, kind="ExternalOutput").ap()

    # tanh chunks of k-tiles: triples + remainder (bigger ACT instructions
    # amortize the fixed SBUF/PSUM access latency per instruction)
    CW = 3
    chunks = [(i, min(i + CW, nkc)) for i in range(0, nkc, CW)]

    with tile.TileContext(nc) as tc:
        with (
            tc.tile_pool(name="resident", bufs=1) as res_pool,
            tc.tile_pool(name="pt", bufs=3) as pt_pool,
            tc.tile_pool(name="onorm", bufs=9) as onorm_pool,
            tc.tile_pool(name="rspool", bufs=1) as rs_pool,
            tc.tile_pool(name="yout", bufs=2) as y_pool,
            tc.tile_pool(name="qk_ps", bufs=2, space="PSUM") as qk_ps,
            tc.tile_pool(name="pvj_ps", bufs=2, space="PSUM") as pvj_ps,
        ):
            # Spread resident loads over queues; first QK needs only kT pair 0
            # and the first q-tile, so those go first on the sync queue.
            kT_sb = res_pool.tile([128, 4, kp], F16, tag="kT")
            nc.sync.dma_start(out=kT_sb[:, 0, 0:256], in_=kT_d[:, 0, 0:256])
            qT_sb = res_pool.tile([128, 4, S], F16, tag="qT")
            nc.sync.dma_start(out=qT_sb[:, 0, 0:QT], in_=qT_d[:, 0, 0:QT])
            nc.sync.dma_start(out=kT_sb[:, 0, 256:kp], in_=kT_d[:, 0, 256:kp])
            nc.sync.dma_start(out=qT_sb[:, 0, QT:S], in_=qT_d[:, 0, QT:S])
            v_sb = res_pool.tile([128, HPC, nkc, DH + 1], F16, tag="v")
            for h in range(HPC):
                nc.gpsimd.dma_start(out=v_sb[:, h], in_=v_d[:, h])
            woT_sb = res_pool.tile([128, 4, E], F16, tag="woT")
            for j in range(1, 4):
                nc.gpsimd.dma_start(out=kT_sb[:, j], in_=kT_d[:, j])
                nc.gpsimd.dma_start(out=qT_sb[:, j], in_=qT_d[:, j])
            nc.gpsimd.dma_start(out=woT_sb, in_=woT_d)

            def emit_proj_eo(onorm_list, qt_idx, eo, final=False):
                if final:
                    # after all QK work: borrow the idle qk pool so the
                    # tail projection double-buffers
                    py_full = qk_ps.tile([128, CW * QT], F32, tag="qk", name="pyf")
                    py = py_full[:, 0:QT]
                else:
                    py = pvj_ps.tile([128, QT], F32, tag="pvj")
                for jj in range(4):
                    nc.tensor.matmul(
                        py,
                        lhsT=woT_sb[:, jj, eo * 128 : (eo + 1) * 128],
                        rhs=onorm_list[jj],
                        start=(jj == 0),
                        stop=(jj == 3),
                    )
                y_t = y_pool.tile([128, QT], F16, tag="y")
                nc.vector.tensor_copy(out=y_t, in_=py)
                nc.sync.dma_start(
                    out=yT_d[
                        eo * 128 : (eo + 1) * 128,
                        qt_idx * QT : (qt_idx + 1) * QT,
                    ],
                    in_=y_t,
                )

            pending = None  # (onorm_tiles, qt) awaiting projection emission
            state = {"onorm_tiles": [], "onorm_cur": None, "pending": None}

            def emit_pv(qt, h, pt_t):
                """PV + normalize for head h of q-tile qt (issued one head
                late so the PE's QK of head h+1 overlaps ACT's exp of h)."""
                half = h % 2
                # PV output [o^T; r] at psum partitions 0-64 (matmul psum
                # base must be 0/32/64). Normalized even-head output lands
                # at partitions 0-63 of the pair tile directly; odd-head
                # output is normalized into a scratch tile and DMA-moved to
                # partitions 64-127 so the projection runs at K=128.
                po_full = pvj_ps.tile([128, QT], F32, tag="pvj", name="po")
                po_t = po_full[0 : DH + 1]
                for kt in range(nkc):
                    nc.tensor.matmul(
                        po_t,
                        lhsT=v_sb[:, h, kt, :],
                        rhs=pt_t[:, kt * QT : (kt + 1) * QT],
                        start=(kt == 0),
                        stop=(kt == nkc - 1),
                    )
                # r lives on psum partition 64; engines are partition-locked,
                # so copy it to SBUF, DMA-move to partition 0, recip there,
                # then gpsimd-broadcast to partitions 0-63.
                rs = rs_pool.tile([DH + 1, QT], F32, tag=f"rs{half}")
                nc.vector.tensor_copy(
                    out=rs[DH : DH + 1, :], in_=po_t[DH : DH + 1, :]
                )
                mv = rs_pool.tile([1, QT], F32, tag=f"mv{half}")
                nc.sync.dma_start(out=mv, in_=rs[DH : DH + 1, :])
                rec1 = rs_pool.tile([1, QT], F32, tag=f"rec{half}")
                nc.vector.reciprocal_approx_fast(out=rec1, in_=mv)
                rb = rs_pool.tile([64, QT], F32, tag=f"rb{half}")
                nc.gpsimd.partition_broadcast(rb, rec1)
                if half == 0:
                    onorm = onorm_pool.tile([128, QT], F16, tag="on")
                    state["onorm_cur"] = onorm
                    nc.vector.tensor_mul(out=onorm[0:64], in0=po_t[0:DH, :], in1=rb)
                else:
                    onorm = state["onorm_cur"]
                    o_scr = rs_pool.tile([64, QT], F16, tag="oscr")
                    nc.vector.tensor_mul(out=o_scr, in0=po_t[0:DH, :], in1=rb)
                    nc.sync.dma_start(out=onorm[64:128], in_=o_scr)
                    state["onorm_tiles"].append(onorm)
                if h == HPC - 1:
                    state["pending"] = (state["onorm_tiles"], qt)
                    state["onorm_tiles"] = []

            prev = None  # (qt, h, pt_t) whose PV is deferred one head
            for qt in [q for _ in range(reps) for q in range(NQT)]:
                for h in range(HPC):
                    j, half = h // 2, h % 2
                    lo = 64 * half
                    pt_t = pt_pool.tile([128, nkc * QT], F16, tag="pt")
                    for c0, c1 in chunks:
                        nk = c1 - c0
                        ps = qk_ps.tile([128, CW * QT], F32, tag="qk", name="ps")
                        for w in range(nk):
                            kt = c0 + w
                            nc.tensor.matmul(
                                ps[:, w * QT : (w + 1) * QT],
                                lhsT=kT_sb[lo : lo + 64, j, kt * 128 : (kt + 1) * 128],
                                rhs=qT_sb[lo : lo + 64, j, qt * QT : (qt + 1) * QT],
                                start=True,
                                stop=True,
                            )
                        nc.scalar.activation(
                            out=pt_t[:, c0 * QT : c1 * QT],
                            in_=ps[:, 0 : nk * QT],
                            func=mybir.ActivationFunctionType.Tanh,
                            scale=scale_eff,
                        )
                    nc.scalar.activation(
                        out=pt_t,
                        in_=pt_t,
                        func=mybir.ActivationFunctionType.Exp,
                        scale=gain,
                    )
                    if prev is not None:
                        emit_pv(*prev)
                    prev = (qt, h, pt_t)
                    # Spread the previous q-tile's projection one eo-block
                    # per head stage so it overlaps ACT work instead of
                    # stalling the PE in one burst.
                    if state["pending"] is not None and h >= 1:
                        ol, pqt = state["pending"]
                        emit_proj_eo(ol, pqt, h - 1)
                        if h == HPC - 1:
                            emit_proj_eo(ol, pqt, 7)
                            state["pending"] = None
            emit_pv(*prev)
            ol, pqt = state["pending"]
            for eo in range(8):
                emit_proj_eo(ol, pqt, eo, final=True)

    nc.compile()
    return nc


def _get_nc(scale_eff: float, gain: float, nkc: int):
    key = (round(scale_eff, 12), round(gain, 12), nkc)
    if key not in _CACHE:
        _CACHE[key] = _build(scale_eff, gain, nkc)
    return _CACHE[key]


def _prep_core_inputs(query, key, value, mask, W_out, nkc):
    """Host-side mask compression + sharding + layout. List of 8 in_maps."""
    kp = nkc * 128
    keep = ~mask[:, 0, :]  # [B, S]; True in mask = drop

    per_batch = []
    for b in range(B):
        idx = np.flatnonzero(keep[b])
        nk = len(idx)
        k_g = np.zeros((kp, E), dtype=np.float32)
        k_g[:nk] = key[b][idx]
        v_g = np.zeros((kp, E), dtype=np.float32)
        v_g[:nk] = value[b][idx]
        ones_g = np.zeros((kp, 1, 1), dtype=np.float32)
        ones_g[:nk] = 1.0
        per_batch.append((k_g, v_g, ones_g))

    in_maps = []
    for c in range(N_CORES):
        b, hh = c // 2, c % 2
        hsl = slice(8 * hh, 8 * hh + 8)
        k_g, v_g, ones_g = per_batch[b]

        k4 = k_g.reshape(kp, H, DH)[:, hsl, :]  # [kp, 8, 64]
        kT = np.ascontiguousarray(
            k4.transpose(1, 2, 0).reshape(4, 128, kp).transpose(1, 0, 2)
        ).astype(np.float16)
        q4 = query[b].reshape(S, H, DH)[:, hsl, :]
        qT = np.ascontiguousarray(
            q4.transpose(1, 2, 0).reshape(4, 128, S).transpose(1, 0, 2)
        ).astype(np.float16)
        woT = np.ascontiguousarray(
            W_out.reshape(E, H, DH)[:, hsl, :]
            .transpose(1, 2, 0)
            .reshape(4, 128, E)
            .transpose(1, 0, 2)
        ).astype(np.float16)  # [128, 4, E], head pairs stacked on partitions

        v4 = v_g.reshape(kp, H, DH)[:, hsl, :]  # [kp, 8, 64]
        ones_b = np.broadcast_to(ones_g, (kp, HPC, 1))
        aug = np.concatenate([v4, ones_b], axis=2)  # [kp, 8, 65]
        vA = np.ascontiguousarray(
            aug.reshape(nkc, 128, HPC, DH + 1).transpose(1, 2, 0, 3)
        ).astype(np.float16)

        in_maps.append({"kT": kT, "qT": qT, "vA": vA, "woT": woT})
    return in_maps


def kernel(query, key, value, mask, W_out, scale, log_C) -> np.ndarray:
    query = np.asarray(query, dtype=np.float32)
    key = np.asarray(key, dtype=np.float32)
    value = np.asarray(value, dtype=np.float32)
    mask = np.asarray(mask)
    W_out = np.asarray(W_out, dtype=np.float32)
    scale_eff = float(np.asarray(scale)) / math.sqrt(DH)
    gain = float(np.exp(np.float64(np.asarray(log_C))))

    keep_counts = (~mask[:, 0, :]).sum(axis=1)
    nkc = max(1, int(math.ceil(int(keep_counts.max()) / 128)))

    nc = _get_nc(scale_eff, gain, nkc)
    in_maps = _prep_core_inputs(query, key, value, mask, W_out, nkc)

    old = nc.m
    nc.m = get_hw_module(nc.m)
    try:
        res = bass_utils.run_bass_kernel_spmd(
            nc, in_maps, core_ids=list(range(N_CORES))
        )
    finally:
        nc.m = old

    out = np.empty((B, S, E), dtype=np.float32)
    for b in range(B):
        yT = res.results[2 * b]["yT"].astype(np.float32) + res.results[
            2 * b + 1
        ]["yT"].astype(np.float32)
        out[b] = yT.T
    return out
